# revision 67
# baseline (speedup 1.0000x reference)
"""Causal attention (anti-causal masked, faithful to reference) on 8 TRN2 cores.

Sharding: data-parallel over batch (2) x tensor-parallel over heads (16 -> 4
groups of 4 heads). Core c handles batch c//4, heads [ (c%4)*4, (c%4)*4+4 ).

Per-core design (hardcoded for B=2, S=2048, D=1024, H=16, dh=64):
  - Host pre-packs inputs into [128, kc, .] walls so each lands in few large
    DMAs (the DMA engine pool is serialized); x arrives in 4 s-chunk DMAs so
    the first projection starts ~6us in.
  - QT/KT computed transposed [c, s]; PSUM->SBUF copy with fused
    per-partition bias on DVE.  V computed natural [s, c] (+ a ones column
    per head) with a host-prebroadcast bias added on DVE.
  - Scores computed transposed per (head, k-tile j) over the exact live
    extent q < 128*(j+1) (the reference keeps only strictly-future keys,
    k > q); exp on ACT with fused scale 1/4 and bias -4 into fp16 (the
    shift guards fp16 overflow and cancels in the softmax division); the
    half-masked diagonal 128x128 block is zeroed multiplicatively on DVE.
  - PV in natural layout: out[q, d] accumulated over k-tiles with
    lhsT = P^T block [128k, 128q], rhs = V-aug [128k, 65].  The cost model
    charges matmuls by OUT free size only, so this costs 65 rows per
    (head, q-tile, k-tile) instead of the 512 a [d, q]-layout PV pays; the
    V ones column makes psum column 64 the softmax denominator for free.
    One reciprocal + 4 tensor_scalar_mul per q-tile normalize on DVE.
  - Last query row (all keys masked -> reference softmax degenerates to the
    uniform average of V) handled uniformly: ex[., 2047] = 1 for the last
    k-tile plus a zeros-except-last-column lhsT accumulated against every
    other V tile in PV(qt=15), so row 2047 = mean(V) with denominator 2048
    through the standard normalize path.
  - Scheduling: the scores PSUM ring (3 x [128,1024] tiles) executes in
    emission order, so emission order is the schedule.  h0/h1 scores (ct0
    weights only) start right after the ct0 projections; ct1 projection
    groups are split into 4-matmul filler pieces popped between score
    chunks; the loop pairs descending h2/h3 k-tiles with ascending h0/h1
    k-tiles so ACT (exp) work per step is constant; V tiles and PV pieces
    backfill PE between exp-ring waits.  PSUM: 6 banks scores + 2 banks
    V/PV.  SBUF: x/w walls live on a right-side stack; the h2/h3 j>=14
    exp pool reuses the released QK-wall space.
"""

import numpy as np

import concourse.bass as bass
import concourse.tile as tile
from concourse import bacc, mybir
from concourse.bass_utils import run_bass_kernel_spmd

F32 = mybir.dt.float32
F16 = mybir.dt.float16
AF = mybir.ActivationFunctionType

B, S, D, H, DH = 2, 2048, 1024, 16, 64
N_CORES = 8
HPC = 4            # heads per core
C = HPC * DH       # channels per core (256)
KC = D // 128      # contraction chunks (8)
NT = S // 128      # 128-tiles along sequence (16)
CW = 1024          # scores/exp chunk width (2 PSUM banks)
EXP_SHIFT = 4.0    # exp(s/4 - 4): keeps fp16 P in range; cancels in division

_CACHE = {}


def _ext(j):
    """Live q extent for k-tile j (strict k > q mask); j=15 padded to 2048
    so the dead last column can carry the uniform-last-row ones."""
    return S if j == NT - 1 else 128 * (j + 1)


def _emit(tc, xw, wqk, wvw, bqk, bvf, out):
    nc = tc.nc
    DT = F16

    const_p = tc.alloc_tile_pool(name="const", bufs=1)
    xw_p = tc.alloc_tile_pool(name="xw", bufs=1, side="right")
    wqk2_p = tc.alloc_tile_pool(name="wqk2", bufs=1, side="right")
    wqk_p = tc.alloc_tile_pool(name="wqk", bufs=1, side="right")
    qk_p = tc.alloc_tile_pool(name="qk", bufs=4)
    v_p = tc.alloc_tile_pool(name="v", bufs=NT)
    ex_a = tc.alloc_tile_pool(name="exa", bufs=HPC)      # j <= 13, all heads
    ex_ha = tc.alloc_tile_pool(name="exha", bufs=2)      # j = 14/15, h0/h1
    rc_p = tc.alloc_tile_pool(name="rc", bufs=3)
    os_p = tc.alloc_tile_pool(name="os", bufs=3)
    ps_big = tc.alloc_tile_pool(name="psbig", bufs=3, space="PSUM")
    ps_pv = tc.alloc_tile_pool(name="pspv", bufs=2, space="PSUM")
    ex_hb = None  # j = 14/15, h2/h3 -- allocated after wqk wall release

    # ---- constants (no DMA deps; fills t=0 on DVE/Pool) ----
    # strict lower-triangle keep mask: (p, f) = 1 iff f < p
    mask = const_p.tile([128, 128], DT, tag="mask")
    nc.vector.memset(mask[:], 1.0)
    nc.gpsimd.affine_select(
        out=mask[:],
        in_=mask[:],
        compare_op=mybir.AluOpType.is_ge,
        fill=0.0,
        base=-1,
        pattern=[[-1, 128]],
        channel_multiplier=1,
    )
    # zeros except last column = 1 (uniform last-row accumulator)
    zcol = const_p.tile([128, 128], DT, tag="zcol")
    nc.vector.memset(zcol[:], 0.0)
    nc.vector.memset(zcol[:, 127:128], 1.0)
    expb = const_p.tile([128, 1], F32, tag="expb")
    nc.vector.memset(expb[:], -EXP_SHIFT)

    # ---- input DMAs (ordered for earliest PE start) ----
    wall = wqk_p.tile([128, KC, 384], DT, tag="wqk")
    nc.sync.dma_start(wall[:], wqk[:, :, 0:384])

    xt = xw_p.tile([128, KC, S], DT, tag="xt")
    wv = xw_p.tile([128, KC, C], DT, tag="wv")
    nc.sync.dma_start(xt[:, :, 0:512], xw[:, :, 0:512])
    bcol = const_p.tile([128, 4], F32, tag="bcol")
    nc.sync.dma_start(bcol[:], bqk[:, :])
    nc.sync.dma_start(xt[:, :, 512:1024], xw[:, :, 512:1024])
    nc.sync.dma_start(wv[:], wvw[:, :, :])
    bvt = const_p.tile([128, C], F32, tag="bvt")
    nc.sync.dma_start(bvt[:], bvf[:, :])
    nc.sync.dma_start(xt[:, :, 1024:1536], xw[:, :, 1024:1536])
    nc.sync.dma_start(xt[:, :, 1536:2048], xw[:, :, 1536:2048])
    wall2 = wqk2_p.tile([128, KC, 128], DT, tag="wqk2")
    nc.sync.dma_start(wall2[:], wqk[:, :, 384:512])
    bvt3 = bvt.rearrange("p (h c) -> p h c", h=HPC)

    # ---- projections: QT/KT transposed [c, s]; copy+bias on DVE ----
    QT = [qk_p.tile([128, S], DT, tag="qkt", name=f"QT{i}") for i in range(2)]
    KT = [qk_p.tile([128, S], DT, tag="qkt", name=f"KT{i}") for i in range(2)]

    def _wslice(ct, tsel, kc):
        if ct == 1 and tsel == 1:
            return wall2[:, kc, 0:128]
        off = ct * 256 + tsel * 128
        return wall[:, kc, off:off + 128]

    def proj_qk(ct, sc, tsel):
        dst = QT if tsel == 0 else KT
        bc = tsel * 2 + ct
        ps = ps_big.tile([128, CW], F32, tag="st")
        for kc in range(KC):
            nc.tensor.matmul(ps[:, 0:512],
                             _wslice(ct, tsel, kc),
                             xt[:, kc, sc * 512:(sc + 1) * 512],
                             start=(kc == 0), stop=(kc == KC - 1))
        nc.vector.tensor_scalar_add(dst[ct][:, sc * 512:(sc + 1) * 512],
                                    ps[:, 0:512], bcol[:, bc:bc + 1])

    # ---- V natural [s, c] + ones col per head (augmented rhs for PV) ----
    Vg = [None] * NT

    def emit_v(si):
        s_sl = slice(si * 128, (si + 1) * 128)
        ps = ps_pv.tile([128, HPC * (DH + 1)], F32, tag="pv")
        for kc in range(KC):
            nc.tensor.matmul(ps[:, 0:C], xt[:, kc, s_sl], wv[:, kc, :],
                             start=(kc == 0), stop=(kc == KC - 1))
        vt = v_p.tile([128, HPC * (DH + 1)], DT, tag="vg", name=f"vg{si}")
        vt3 = vt.rearrange("p (h c) -> p h c", h=HPC)
        nc.vector.memset(vt3[:, :, DH:DH + 1], 1.0)
        ps3 = ps[:, 0:C].rearrange("p (h c) -> p h c", h=HPC)
        nc.vector.tensor_add(vt3[:, :, 0:DH], ps3[:, :, :], bvt3[:, :, :])
        Vg[si] = vt

    # ---- scores + exp for one (head, k-tile) ----
    EX = [[None] * NT for _ in range(HPC)]
    fillers = []  # pending PE filler emitters (V / PV pieces)
    pop_ctl = {"every": 1, "tick": 0}

    def pop_filler():
        if fillers:
            fillers.pop(0)()

    def chunk_pop():
        pop_ctl["tick"] += 1
        if pop_ctl["tick"] % pop_ctl["every"] == 0:
            pop_filler()

    def scores_exp(h, j):
        ct, po = h // 2, (h % 2) * 64
        E = _ext(j)
        if j >= 14:
            pool = ex_ha if h < 2 else ex_hb
        else:
            pool = ex_a
        ex = pool.tile([128, E], DT, tag=f"ex{j}", name=f"ex{h}_{j}")
        for c0 in range(0, E, CW):
            cw = min(CW, E - c0)
            st = ps_big.tile([128, CW], F32, tag="st")
            for p0 in range(0, cw, 512):
                pw = min(512, cw - p0)
                nc.tensor.matmul(st[:, p0:p0 + pw],
                                 KT[ct][po:po + 64,
                                        j * 128:(j + 1) * 128],
                                 QT[ct][po:po + 64, c0 + p0:c0 + p0 + pw],
                                 start=True, stop=True)
            nc.scalar.activation(out=ex[:, c0:c0 + cw], in_=st[:, 0:cw],
                                 func=AF.Exp, scale=0.25, bias=expb[:])
            chunk_pop()
        # zero the masked (k <= q) half of the diagonal 128x128 block
        dq = j * 128
        nc.vector.tensor_mul(ex[:, dq:dq + 128], ex[:, dq:dq + 128],
                             mask[:])
        if j == NT - 1:
            # uniform last row: ones P column -> mean(V), denom 2048
            nc.vector.memset(ex[:, S - 1:S], 1.0)
        EX[h][j] = (ex, 0)

    # ---- PV for one q-tile: out[q, d] over all live k-tiles, 4 heads ----
    def _exsl(h, jp, qt):
        t, base = EX[h][jp]
        return t[:, base + qt * 128:base + (qt + 1) * 128]

    def pv_head(pv3, qt, h):
        if qt == NT - 1:
            nc.tensor.matmul(pv3[:, h, :], _exsl(h, qt, qt),
                             Vg[qt].rearrange("p (h c) -> p h c",
                                              h=HPC)[:, h, :],
                             start=True, stop=False)
            for jp in range(NT - 1):
                nc.tensor.matmul(pv3[:, h, :], zcol[:],
                                 Vg[jp].rearrange("p (h c) -> p h c",
                                                  h=HPC)[:, h, :],
                                 start=False, stop=(jp == NT - 2))
        else:
            for jp in range(qt, NT):
                nc.tensor.matmul(pv3[:, h, :], _exsl(h, jp, qt),
                                 Vg[jp].rearrange("p (h c) -> p h c",
                                                  h=HPC)[:, h, :],
                                 start=(jp == qt), stop=(jp == NT - 1))

    def pv_norm(pv, pv3, qt):
        q_sl = slice(qt * 128, (qt + 1) * 128)
        rc = rc_p.tile([128, HPC], F32, tag="rc")
        rc3 = rc.rearrange("p (a b) -> p a b", b=1)
        nc.vector.reciprocal(rc3[:, :, :], pv3[:, :, DH:DH + 1])
        os = os_p.tile([128, C], F32, tag="os")
        os3 = os.rearrange("p (h c) -> p h c", h=HPC)
        for h in range(HPC):
            nc.vector.tensor_scalar_mul(os3[:, h, :], pv3[:, h, 0:DH],
                                        rc[:, h:h + 1])
        nc.sync.dma_start(out[q_sl, :], os[:])

    def emit_pv(qt):
        pv = ps_pv.tile([128, HPC * (DH + 1)], F32, tag="pv")
        pv3 = pv.rearrange("p (h c) -> p h c", h=HPC)
        for h in range(HPC):
            pv_head(pv3, qt, h)
        pv_norm(pv, pv3, qt)

    def push_proj(ct, sc, tsel):
        holder = {}
        dst = QT if tsel == 0 else KT
        bc = tsel * 2 + ct

        def piece(part):
            if "ps" not in holder:
                holder["ps"] = ps_big.tile([128, CW], F32, tag="st",
                                           name=f"pj{ct}{sc}{tsel}")
            ps = holder["ps"]
            for kc in range(part * 4, part * 4 + 4):
                nc.tensor.matmul(ps[:, 0:512],
                                 _wslice(ct, tsel, kc),
                                 xt[:, kc, sc * 512:(sc + 1) * 512],
                                 start=(kc == 0), stop=(kc == KC - 1))
            if part == 1:
                nc.vector.tensor_scalar_add(
                    dst[ct][:, sc * 512:(sc + 1) * 512], ps[:, 0:512],
                    bcol[:, bc:bc + 1])
        fillers.append(lambda: piece(0))
        fillers.append(lambda: piece(1))

    def push_pv(qt):
        holder = {}

        def piece(h):
            if "pv" not in holder:
                holder["pv"] = ps_pv.tile([128, HPC * (DH + 1)], F32,
                                          tag="pv", name=f"pv{qt}")
                holder["pv3"] = holder["pv"].rearrange("p (h c) -> p h c",
                                                       h=HPC)
            if h is None:
                pv_norm(holder["pv"], holder["pv3"], qt)
            else:
                pv_head(holder["pv3"], qt, h)
        for h in range(HPC):
            fillers.append(lambda h=h: piece(h))
        fillers.append(lambda: piece(None))

    # ---- software-pipelined emission ----
    # The scores/exp PSUM ring executes in emission order, so the emission
    # order largely IS the schedule.  h0/h1 scores (ct0-only) start right
    # after the ct0 projections; ct1 projection groups slot between them so
    # h2/h3 exps are ready by their turn.  V tiles and PV pieces are queued
    # as fillers and popped between score chunks so the PE stream always
    # has independent work next to an exp-ring wait.
    proj_qk(0, 0, 0)
    proj_qk(0, 1, 0)
    proj_qk(0, 2, 0)
    proj_qk(0, 0, 1)
    emit_v(0)
    emit_v(1)
    proj_qk(0, 3, 0)
    proj_qk(0, 3, 1)
    scores_exp(0, 15)
    scores_exp(1, 15)
    scores_exp(0, 14)
    scores_exp(1, 14)
    push_proj(0, 2, 1)
    push_proj(0, 1, 1)
    ct1q = [(1, sc, t) for sc in range(4) for t in (0, 1)]
    for i, j in enumerate((13, 13, 12, 12, 11, 11, 10, 10, 9, 9, 8, 8)):
        if i < 8:
            push_proj(*ct1q[i])
        scores_exp(i % 2, j)
    while fillers:
        pop_filler()
    wqk_p.release()
    ex_hb = tc.alloc_tile_pool(name="exhb", bufs=2, side="right")
    scores_exp(2, 15)
    scores_exp(3, 15)
    fillers.append(lambda: emit_v(15))
    fillers.append(lambda: emit_v(14))
    scores_exp(2, 14)
    scores_exp(3, 14)
    vq = 13
    for j in range(13, -1, -1):
        for _ in range(2):
            if vq >= 2:
                fillers.append(lambda si=vq: emit_v(si))
                vq -= 1
        j01 = 13 - j
        scores_exp(2, j)
        if j01 <= 7:
            scores_exp(0, j01)
        scores_exp(3, j)
        if j01 <= 7:
            scores_exp(1, j01)
        push_pv(j + 1)
        if j == 7:
            push_pv(15)
        while len(fillers) > (5 if j > 4 else 1):
            pop_filler()
    while fillers:
        pop_filler()
    emit_pv(0)

    ex_hb.release()

    for p in (os_p, rc_p, ex_ha, ex_a, v_p, qk_p, const_p, wqk2_p,
              xw_p, ps_pv, ps_big):
        p.release()


def _build():
    if "nc" in _CACHE:
        return _CACHE["nc"]
    nc = bacc.Bacc("TRN2", target_bir_lowering=False, debug=False,
                   num_devices=N_CORES)
    xw = nc.dram_tensor("xw", [128, KC, S], F16, kind="ExternalInput").ap()
    wqk = nc.dram_tensor("wqk", [128, KC, 512], F16,
                         kind="ExternalInput").ap()
    wvw = nc.dram_tensor("wvw", [128, KC, C], F16, kind="ExternalInput").ap()
    bqk = nc.dram_tensor("bqk", [128, 4], F32, kind="ExternalInput").ap()
    bvf = nc.dram_tensor("bvf", [128, C], F32, kind="ExternalInput").ap()
    out = nc.dram_tensor("out", [S, C], F32, kind="ExternalOutput").ap()
    with tile.TileContext(nc) as tc:
        _emit(tc, xw, wqk, wvw, bqk, bvf, out)
    nc.compile()
    _CACHE["nc"] = nc
    return nc


def _wall(wT):
    """[D, n] -> [128, KC, n] with [p, kc, c] = wT[128*kc + p, c]."""
    n = wT.shape[1]
    return np.ascontiguousarray(
        wT.reshape(KC, 128, n).transpose(1, 0, 2)).astype(np.float16)


def make_in_maps(x, Wq, bq, Wk, bk, Wv, bv):
    in_maps = []
    for c in range(N_CORES):
        b, g = c // HPC, c % HPC
        cols = slice(g * C, (g + 1) * C)
        xT = np.ascontiguousarray(x[b].T)
        wqT, wkT = Wq[cols, :].T, Wk[cols, :].T
        wqkT = np.concatenate([wqT[:, 0:128], wkT[:, 0:128],
                               wqT[:, 128:256], wkT[:, 128:256]], axis=1)
        bq_c, bk_c = bq[cols], bk[cols]
        bcol = np.stack([bq_c[0:128], bq_c[128:256],
                         bk_c[0:128], bk_c[128:256]], axis=1)
        in_maps.append({
            "xw": _wall(xT),
            "wqk": _wall(wqkT),
            "wvw": _wall(Wv[cols, :].T),
            "bqk": bcol.astype(np.float32),
            "bvf": np.ascontiguousarray(
                np.broadcast_to(bv[cols], (128, C))).astype(np.float32),
        })
    return in_maps


def assemble(results):
    out = np.empty((B, S, D), np.float32)
    for c in range(N_CORES):
        b, g = c // HPC, c % HPC
        out[b, :, g * C:(g + 1) * C] = results[c]["out"]
    return out


def kernel(x, Wq, bq, Wk, bk, Wv, bv):
    nc = _build()
    in_maps = make_in_maps(x, Wq, bq, Wk, bk, Wv, bv)
    res = run_bass_kernel_spmd(nc, in_maps, core_ids=list(range(N_CORES)))
    return assemble(res.results)
